# revision 5
# baseline (speedup 1.0000x reference)
"""Causal self-attention (B=4, T=2048, D=1024, H=16, head_dim=64) on 8 TRN2
NeuronCores — v4: all-bf16 datapath (4x DVE modes, half DMA), x and all
weights resident in SBUF (loaded once), attention for pair p overlapped with
q/k projection for pair p+1, multiplicative causal masks on GpSimd.

Sharding: core c handles batch b = c//2 and head-half hh = c%2 (8 heads);
host sums the two partial output projections per batch.
"""
import os
import sys

sys.path.insert(0, "/opt/trn_rl_repo")

import numpy as np

import concourse.bass as bass
import concourse.mybir as mybir
import concourse.tile as tile
from concourse import bacc
from concourse.bass_utils import run_bass_kernel_spmd

F32 = mybir.dt.float32
BF16 = mybir.dt.bfloat16
EXP = mybir.ActivationFunctionType.Exp
MUL = mybir.AluOpType.mult

B, T, DIM, HEADS, HD = 4, 2048, 1024, 16, 64
THETA = 10000.0
NCORES = 8


def _consts():
    freqs = 1.0 / THETA ** (np.arange(0, HD, 2, dtype=np.float32) / HD)
    t = np.arange(T, dtype=np.float32)
    ang = t[None, :] * freqs[np.arange(128) % 32, None]
    cosT = np.cos(ang).astype(np.float32)
    sinT = np.sin(ang).astype(np.float32)

    P = np.zeros((128, 128), dtype=np.float32)
    for i in range(128):
        base, il = (i // 64) * 64, i % 64
        if il < 32:
            P[i, base + il + 32] = -1.0
        else:
            P[i, base + il - 32] = 1.0
    PT = P.T.copy()

    k = np.arange(128)[:, None]
    q = np.arange(128)[None, :]
    tri01 = (k <= q).astype(np.float32)
    tri01j3 = np.zeros((128, 256), dtype=np.float32)
    tri01j3[:, 128:] = tri01
    return cosT, sinT, PT, tri01, tri01j3


def _build(repeat=1):
    nc = bacc.Bacc("TRN2", target_bir_lowering=False, debug=False)

    xT = nc.dram_tensor("xT", [DIM, T], BF16, kind="ExternalInput")
    wqk = nc.dram_tensor("wqk", [DIM, 1024], BF16, kind="ExternalInput")
    wv = nc.dram_tensor("wv", [DIM, 512], BF16, kind="ExternalInput")
    wo = nc.dram_tensor("wo", [512, DIM], BF16, kind="ExternalInput")
    cosT_d = nc.dram_tensor("cosT", [128, T], BF16, kind="ExternalInput")
    sinT_d = nc.dram_tensor("sinT", [128, T], BF16, kind="ExternalInput")
    PT_d = nc.dram_tensor("PT", [128, 128], BF16, kind="ExternalInput")
    tri_d = nc.dram_tensor("tri", [128, 128], BF16, kind="ExternalInput")
    trij3_d = nc.dram_tensor("trij3", [128, 256], BF16, kind="ExternalInput")
    ones_d = nc.dram_tensor("ones", [128, 128], BF16, kind="ExternalInput")
    outp = nc.dram_tensor("outp", [T, DIM], BF16, kind="ExternalOutput")

    xr = xT.rearrange("(c p) t -> p c t", p=128)

    with tile.TileContext(nc) as tc:
      for _rep in range(repeat):
        with (
            tc.tile_pool(name="glob", bufs=1) as glob,
            tc.tile_pool(name="qk", bufs=2) as qkpool,
            tc.tile_pool(name="p2", bufs=1) as p2,
            tc.tile_pool(name="p2dram", bufs=8, space="DRAM") as p2dram,
        ):
          att = p2.tile([128, 4, T], BF16)  # att_norm^T [attdim, t]
          with (
            tc.tile_pool(name="xw", bufs=1) as xw,
            tc.tile_pool(name="p1t", bufs=2) as p1t,
            tc.tile_pool(name="p2pt", bufs=2) as p2pt,
            tc.tile_pool(name="p2u", bufs=2) as p2u,
            tc.tile_pool(name="p2n", bufs=2) as p2n,
            tc.tile_pool(name="p1rot", bufs=1, space="PSUM") as p1rot,
          ):
            pools = {}
            x_sb = xw.tile([128, 8, T], BF16)
            wqk_sb = xw.tile([128, 8, 1024], BF16)
            wv_sb = xw.tile([128, 8, 512], BF16)
            wo_sb = xw.tile([128, 4, 1024], BF16)
            v_aug = glob.tile([128, 16, 8, 65], BF16)
            tri_sb = glob.tile([128, 128], BF16)
            trij3_sb = glob.tile([128, 256], BF16)
            cos_sb = glob.tile([128, T], BF16)
            sin_sb = glob.tile([128, T], BF16)
            PT_sb = glob.tile([128, 128], BF16)
            # startup DMA order matters: phase A needs x chunk 0 + wv first,
            # then pair-0's wqk columns (m=0 and m=4) + rope tables; the bulk
            # of wqk, masks, ones and wo are consumed later.
            wqkr = wqk.rearrange("(c p) m -> p c m", p=128)
            nc.sync.dma_start(
                out=x_sb[:, :, 0:512], in_=xr[:, :, 0:512])
            nc.sync.dma_start(
                out=wv_sb, in_=wv.rearrange("(c p) m -> p c m", p=128))
            nc.sync.dma_start(out=wqk_sb[:, :, 0:128], in_=wqkr[:, :, 0:128])
            nc.sync.dma_start(out=wqk_sb[:, :, 512:640], in_=wqkr[:, :, 512:640])
            nc.sync.dma_start(out=cos_sb, in_=cosT_d[:])
            nc.sync.dma_start(out=sin_sb, in_=sinT_d[:])
            nc.sync.dma_start(out=PT_sb, in_=PT_d[:])
            for xn in range(1, 4):
                nc.sync.dma_start(
                    out=x_sb[:, :, xn * 512:(xn + 1) * 512],
                    in_=xr[:, :, xn * 512:(xn + 1) * 512])
            nc.sync.dma_start(out=wqk_sb[:, :, 128:512], in_=wqkr[:, :, 128:512])
            nc.sync.dma_start(out=wqk_sb[:, :, 640:1024], in_=wqkr[:, :, 640:1024])
            nc.sync.dma_start(out=tri_sb, in_=tri_d[:])
            nc.sync.dma_start(out=trij3_sb, in_=trij3_d[:])
            nc.sync.dma_start(
                out=v_aug[:, :, :, 64:65],
                in_=ones_d.rearrange("p (a b o) -> p a b o", a=16, o=1),
            )
            nc.sync.dma_start(
                out=wo_sb, in_=wo.rearrange("(c p) m -> p c m", p=128))

            qk_tiles = {}  # pair -> (q_tile, k_tile), each [128, T] bf16

            def proj_unit(m, n):
                """q (m<4) / k (m>=4) projection rows 128m, T-block n, + rope."""
                pair = m % 4
                dest = qk_tiles[pair][0 if m < 4 else 1]
                ncol = slice(n * 512, (n + 1) * 512)
                mcol = slice(m * 128, (m + 1) * 128)
                ps = pools["proj"].tile([128, 512], F32, tag="proj", name="ps")
                for k in range(8):
                    nc.tensor.matmul(
                        ps[:], wqk_sb[:, k, mcol], x_sb[:, k, ncol],
                        start=(k == 0), stop=(k == 7),
                    )
                raw = p1t.tile([128, 512], BF16, tag="raw", bufs=1)
                nc.vector.tensor_copy(raw[:], ps[:])
                rotp = p1rot.tile([128, 512], F32)
                nc.tensor.matmul(rotp[:], PT_sb[:], raw[:], start=True, stop=True)
                t2 = p1t.tile([128, 512], BF16, tag="t2", bufs=1)
                nc.vector.tensor_mul(t2[:], rotp[:], sin_sb[:, ncol])
                nc.vector.tensor_mul(dest[:, ncol], raw[:], cos_sb[:, ncol])
                nc.vector.tensor_add(dest[:, ncol], dest[:, ncol], t2[:])

            def qk_stream(pair):
                """Closures: allocate pair's q/k tiles and emit its 8 units."""
                out = []

                def alloc():
                    qt = qkpool.tile([128, T], BF16, tag="q", name=f"qt{pair}")
                    kt = qkpool.tile([128, T], BF16, tag="k", name=f"kt{pair}")
                    qk_tiles[pair] = (qt, kt)
                out.append(alloc)
                for m in (pair, pair + 4):
                    for n in range(4):
                        out.append(lambda m=m, n=n: proj_unit(m, n))
                return out

            # ---- Phase A: v projection (all T) + pair 0 q/k ----
            with tc.tile_pool(name="psA", bufs=4, space="PSUM") as psA:
                pools["proj"] = psA
                qk_tiles[0] = (qkpool.tile([128, T], BF16, tag="q", name="qt0"),
                               qkpool.tile([128, T], BF16, tag="k", name="kt0"))
                for n in range(4):
                    for ts in range(4):
                        psv = pools["proj"].tile([128, 512], F32, tag="proj", name="psv")
                        for k in range(8):
                            nc.tensor.matmul(
                                psv[:],
                                x_sb[:, k, n * 512 + ts * 128:n * 512 + (ts + 1) * 128],
                                wv_sb[:, k, :], start=(k == 0), stop=(k == 7),
                            )
                        nc.vector.tensor_copy(
                            v_aug[:, n * 4 + ts, :, 0:64],
                            psv.rearrange("p (h d) -> p h d", h=8),
                        )
                    for m in (0, 4):
                        proj_unit(m, n)

            # ---- Phase B: attention(p) interleaved with projections(p+1);
            # slot 3 also emits the output projection per query block ----
            with (tc.tile_pool(name="p3ob", bufs=2) as p3ob,
                  tc.tile_pool(name="p2st", bufs=2, space="PSUM") as p2st,
                  tc.tile_pool(name="p2o", bufs=1, space="PSUM") as ps_out,
                  tc.tile_pool(name="p1ps", bufs=1, space="PSUM") as p1ps):
                pools["proj"] = p1ps

                def p3_chunk(qb):
                    """Output projection for the 4 T-chunks of query block qb;
                    runs in slot 3 reusing the idle projection PSUM slots."""
                    for i, tcb in enumerate(range(4 * qb, 4 * qb + 4)):
                        for od in range(2):
                            if (i * 2 + od) % 2 == 0:
                                po = p1ps.tile([128, 512], F32, tag="proj",
                                               name="po")
                            else:
                                po = p1rot.tile([128, 512], F32, tag="rotp",
                                                name="po2")
                            for ac in range(4):
                                nc.tensor.matmul(
                                    po[:], att[:, ac, tcb * 128:(tcb + 1) * 128],
                                    wo_sb[:, ac, od * 512:(od + 1) * 512],
                                    start=(ac == 0), stop=(ac == 3),
                                )
                            ob = p3ob.tile([128, 512], BF16, tag="ob")
                            nc.vector.tensor_copy(ob[:], po[:])
                            nc.sync.dma_start(
                                out=outp[tcb * 128:(tcb + 1) * 128,
                                         od * 512:(od + 1) * 512],
                                in_=ob[:],
                            )
                def kc_block(p, qb, kc, nkc, o2):
                    qt, kt = qk_tiles[p]
                    j = kc - 4 * qb
                    c0 = 0 if j < 0 else 128 * j
                    qcol = slice(qb * 512 + c0, (qb + 1) * 512)
                    kcol = slice(kc * 128, (kc + 1) * 128)
                    st = p2st.tile([128, 1024], F32, tag="st")
                    nc.tensor.matmul(st[:, c0:512], kt[0:64, kcol],
                                     qt[0:64, qcol], start=True, stop=True)
                    nc.tensor.matmul(st[:, 512 + c0:1024], kt[64:128, kcol],
                                     qt[64:128, qcol], start=True, stop=True)
                    pt = p2pt.tile([128, 1024], BF16)
                    nc.scalar.activation(pt[:, c0:1024], st[:, c0:1024], EXP,
                                         bias=0.0, scale=0.125)
                    if j >= 0:
                        ms = slice(128 * j, 128 * j + 128)
                        nc.gpsimd.tensor_tensor(pt[:, ms], pt[:, ms],
                                                tri_sb[:], MUL)
                        ms2 = slice(512 + ms.start, 512 + ms.stop)
                        nc.gpsimd.tensor_tensor(pt[:, ms2], pt[:, ms2],
                                                tri_sb[:], MUL)
                    nc.tensor.matmul(
                        o2[:, c0:512], v_aug[:, kc, 2 * p, :], pt[:, c0:512],
                        start=(kc == 0), stop=(kc == nkc - 1),
                        skip_group_check=True,
                    )
                    nc.tensor.matmul(
                        o2[:, 512 + c0:1024], v_aug[:, kc, 2 * p + 1, :],
                        pt[:, 512 + c0:1024], start=(kc == 0),
                        stop=(kc == nkc - 1), skip_group_check=True,
                    )

                def kc_block_j23(p, qb, nkc, o2):
                    """Merged diagonal tail: kc=4qb+2 (j=2, 256 live q cols)
                    and kc=4qb+3 (trimmed to its last 128 q cols) share one
                    st/pt tile and a single full-width exp. Placements keep
                    every matmul output inside one PSUM bank:
                      j2 A [256:512)  j2 B [768:1024)
                      j3 A [0:128)    j3 B [512:640)
                    (the gaps hold stale-but-bounded scores, exp'd and never
                    consumed -- same pattern as the c0-garbage above)."""
                    qt, kt = qk_tiles[p]
                    kc2, kc3 = 4 * qb + 2, 4 * qb + 3
                    q2 = slice(qb * 512 + 256, (qb + 1) * 512)
                    q3 = slice(qb * 512 + 384, (qb + 1) * 512)
                    kcol2 = slice(kc2 * 128, kc2 * 128 + 128)
                    kcol3 = slice(kc3 * 128, kc3 * 128 + 128)
                    st = p2st.tile([128, 1024], F32, tag="st")
                    nc.tensor.matmul(st[:, 256:512], kt[0:64, kcol2],
                                     qt[0:64, q2], start=True, stop=True)
                    nc.tensor.matmul(st[:, 768:1024], kt[64:128, kcol2],
                                     qt[64:128, q2], start=True, stop=True)
                    nc.tensor.matmul(st[:, 0:128], kt[0:64, kcol3],
                                     qt[0:64, q3], start=True, stop=True)
                    nc.tensor.matmul(st[:, 512:640], kt[64:128, kcol3],
                                     qt[64:128, q3], start=True, stop=True)
                    pt = p2pt.tile([128, 1024], BF16)
                    nc.scalar.activation(pt[:], st[:], EXP, bias=0.0,
                                         scale=0.125)
                    for ms in (slice(256, 384), slice(768, 896),
                               slice(0, 128), slice(512, 640)):
                        nc.gpsimd.tensor_tensor(pt[:, ms], pt[:, ms],
                                                tri_sb[:], MUL)
                    nc.tensor.matmul(o2[:, 256:512], v_aug[:, kc2, 2 * p, :],
                                     pt[:, 256:512], start=False, stop=False,
                                     skip_group_check=True)
                    nc.tensor.matmul(o2[:, 768:1024],
                                     v_aug[:, kc2, 2 * p + 1, :],
                                     pt[:, 768:1024], start=False, stop=False,
                                     skip_group_check=True)
                    nc.tensor.matmul(o2[:, 384:512], v_aug[:, kc3, 2 * p, :],
                                     pt[:, 0:128], start=False, stop=True,
                                     skip_group_check=True)
                    nc.tensor.matmul(o2[:, 896:1024],
                                     v_aug[:, kc3, 2 * p + 1, :],
                                     pt[:, 512:640], start=False, stop=True,
                                     skip_group_check=True)

                def att_stream(p):
                    ctx = {}
                    out = []
                    for qb in range(4):
                        nkc = 4 * qb + 4

                        def mk_o2():
                            ctx["o2"] = ps_out.tile([65, 1024], F32, tag="o", name="o2")
                        out.append(mk_o2)
                        for kc in range(nkc - 2):
                            out.append(lambda p=p, qb=qb, kc=kc, nkc=nkc:
                                       kc_block(p, qb, kc, nkc, ctx["o2"]))
                        out.append(lambda p=p, qb=qb, nkc=nkc:
                                   kc_block_j23(p, qb, nkc, ctx["o2"]))

                        def norm(p=p, qb=qb):
                            # decouple: copy unnormalized out + sums to SBUF,
                            # freeing the PSUM accumulator quickly
                            o2 = ctx["o2"]
                            attu = p2u.tile([65, 1024], BF16, tag="attu")
                            nc.vector.tensor_copy(attu[:], o2[:])
                            rsum = p2n.tile([1, 1024], BF16, tag="rsum", bufs=1)
                            with nc.allow_low_precision(
                                    reason="bf16 softmax denom recip"):
                                nc.vector.reciprocal(rsum[:], attu[64:65, :])
                            scr = p2dram.tile([1, 1024], BF16)
                            nc.sync.dma_start(out=scr[:], in_=rsum[:])
                            rbc = p2n.tile([64, 1024], BF16, tag="rbc")
                            sap = scr[:]
                            nc.sync.dma_start(
                                out=rbc[:],
                                in_=bass.AP(tensor=sap.tensor, offset=sap.offset,
                                            ap=[[0, 64], [1, 1024]]),
                            )
                            qcols = slice(qb * 512, (qb + 1) * 512)
                            nc.vector.tensor_mul(att[0:64, p, qcols],
                                                 attu[0:64, 0:512], rbc[:, 0:512])
                            nc.vector.tensor_mul(att[64:128, p, qcols],
                                                 attu[0:64, 512:1024],
                                                 rbc[:, 512:1024])
                        out.append(norm)
                        if p == 3:
                            out.append(lambda qb=qb: p3_chunk(qb))
                    return out

                for p in range(4):
                    a_stream = att_stream(p)
                    q_stream = qk_stream(p + 1) if p < 3 else []
                    na, nq = len(a_stream), len(q_stream)
                    qi = 0
                    for i, fn in enumerate(a_stream):
                        fn()
                        want = ((i + 1) * nq) // na
                        while qi < want:
                            q_stream[qi]()
                            qi += 1

    nc.compile()
    return nc


_NC = {}


def _get_nc(repeat=1):
    if repeat not in _NC:
        _NC[repeat] = _build(repeat)
    return _NC[repeat]


def _bf(a):
    import ml_dtypes
    return np.ascontiguousarray(a.astype(ml_dtypes.bfloat16))


def _in_maps(x, w_qkv, w_out):
    cosT, sinT, PT, tri01, tri01j3 = _consts()
    maps = []
    for c in range(NCORES):
        b, hh = c // 2, c % 2
        wqkm = np.concatenate(
            [w_qkv[:, 512 * hh:512 * hh + 512],
             w_qkv[:, 1024 + 512 * hh:1024 + 512 * hh + 512]], axis=1)
        wvm = w_qkv[:, 2048 + 512 * hh:2048 + 512 * hh + 512]
        wom = w_out[512 * hh:512 * hh + 512, :]
        xTb = x[b].T
        maps.append(dict(xT=_bf(xTb), wqk=_bf(wqkm), wv=_bf(wvm), wo=_bf(wom),
                         cosT=_bf(cosT), sinT=_bf(sinT), PT=_bf(PT),
                         tri=_bf(tri01), trij3=_bf(tri01j3),
                         ones=_bf(np.ones((128, 128), dtype=np.float32))))
    return maps


def kernel(x, w_qkv, w_out):
    x = np.ascontiguousarray(x, dtype=np.float32)
    w_qkv = np.ascontiguousarray(w_qkv, dtype=np.float32)
    w_out = np.ascontiguousarray(w_out, dtype=np.float32)

    nc = _get_nc(int(os.environ.get("KREPEAT", "1")))
    in_maps = _in_maps(x, w_qkv, w_out)
    out = np.empty((B, T, DIM), dtype=np.float32)
    # the shared backend occasionally corrupts a whole execution (outputs
    # ~1e30 instead of ~2); healthy outputs for randn inputs are O(10), so a
    # loose magnitude check detects it reliably and a retry recovers
    for _attempt in range(3):
        r = run_bass_kernel_spmd(nc, in_maps, core_ids=list(range(NCORES)))
        for b in range(B):
            out[b] = (r.results[2 * b]["outp"].astype(np.float32)
                      + r.results[2 * b + 1]["outp"].astype(np.float32))
        if np.isfinite(out).all() and np.abs(out).max() < 1e4:
            break
    kernel.last_results = r
    return out
